# revision 8
# baseline (speedup 1.0000x reference)
"""Jordan RNN layer (B=64, T=512, I=H=O=512) on 8 trn2 NeuronCores.

Data-parallel: batch is sharded 8 ways (8 rows/core); the T=512 sequential
scan runs locally per core with zero collectives.

Math: with a_t = xin_t + W_y y_{t-1} + b_y and y_t = W_out h_t + b_out,
substitute y_{t-1} to get
    a_t = c_t + M tanh(a_{t-1}),   M = W_y @ W_out,
    c_t = xin_t + b_in + b_y + W_y b_out   (t >= 1)
    c_0 = xin_0 + b_in + b_y + W_y last_logits
so the sequential part is ONE 512x512 matvec batch per step; h = tanh(a)
and y = h @ W_out.T + b_out are big parallel matmuls.

On-chip layout is feature-major ([feature, batch*time]) so the scan never
transposes: stationary operand = 16 fp16 M^T chunks (FWL halves their load
time), moving operand = the 8-column tanh activations.
"""

import os
from contextlib import ExitStack

import numpy as np

import concourse.bacc as bacc
import concourse.bass as bass
import concourse.tile as tile
from concourse import mybir
from concourse.bass_utils import run_bass_kernel_spmd

B, T, I, H, O = 64, 512, 512, 512, 512
NCORES = 8
BS = B // NCORES          # batch rows per core
BT = BS * T               # (b, t) columns per core, b-major: col = b*T + t
F32 = mybir.dt.float32
F16 = mybir.dt.float16

LAST_RESULTS = None       # test harness reads exec_time_ns from here


def _build_nc():
    nc = bacc.Bacc()

    emb = nc.dram_tensor("emb", [BT, I], F32, kind="ExternalInput")
    ll = nc.dram_tensor("ll", [BS, O], F32, kind="ExternalInput")
    w_in = nc.dram_tensor("w_in", [H, I], F32, kind="ExternalInput")
    b_in = nc.dram_tensor("b_in", [H], F32, kind="ExternalInput")
    w_y = nc.dram_tensor("w_y", [H, O], F32, kind="ExternalInput")
    b_y = nc.dram_tensor("b_y", [H], F32, kind="ExternalInput")
    w_out = nc.dram_tensor("w_out", [O, H], F32, kind="ExternalInput")
    b_out = nc.dram_tensor("b_out", [O], F32, kind="ExternalInput")
    ident = nc.dram_tensor("ident", [128, 128], F32, kind="ExternalInput")

    h_out = nc.dram_tensor("h_out", [BS, T, H], F32, kind="ExternalOutput")
    y_out = nc.dram_tensor("y_out", [BS, T, O], F32, kind="ExternalOutput")

    KC = H // 128  # 4 feature chunks everywhere

    with tile.TileContext(nc) as tc, tc.tile_pool(name="persist", bufs=1) as per:
        # ---- persistent SBUF ----
        c_sb = per.tile([128, KC * BT], F32, tag="c")        # c^T, col = m*BT + b*T + t
        hT_sb = per.tile([128, KC * BT], F32, tag="h")       # h^T, same layout
        mt_sb = per.tile([128, KC * KC * 128], F16, tag="mt")  # M^T chunk (k,m) at (k*4+m)*128
        woutT = per.tile([128, KC * H], F32, tag="woutT")    # W_out^T chunk k at k*H
        id_sb = per.tile([128, 128], F32, tag="ident")
        bias_sb = per.tile([128, KC], F32, tag="bias")       # b_in + b_y + W_y b_out
        bout_sb = per.tile([128, KC], F32, tag="bout")
        corr_sb = per.tile([128, KC * BS], F32, tag="corr")  # W_y (ll - b_out), per b

        nc.sync.dma_start(id_sb[:], ident[:])

        c_v = c_sb[:].rearrange("p (m b t) -> p m b t", m=KC, b=BS)
        h_v = hT_sb[:].rearrange("p (m b t) -> p m b t", m=KC, b=BS)

        # ---- init: weight transposes, M^T, biases ----
        with (
            tc.tile_pool(name="init", bufs=1) as ini,
            tc.tile_pool(name="init_ps", bufs=2, space="PSUM") as ips,
        ):
            winT = ini.tile([128, KC * H], F32, tag="winT")
            wyT = ini.tile([128, KC * H], F32, tag="wyT")

            for name, dram, dst in (("wi", w_in, winT), ("wy", w_y, wyT),
                                    ("wo", w_out, woutT)):
                wn = ini.tile([128, KC * H], F32, tag="wnat")
                nc.sync.dma_start(
                    wn[:].rearrange("p (r i) -> p r i", r=KC),
                    dram.rearrange("(r p) i -> p r i", p=128))
                for cchunk in range(KC):
                    ps = ips.tile([128, 512], F32, tag="ips")
                    for r in range(KC):
                        nc.tensor.transpose(
                            ps[:, r * 128:(r + 1) * 128],
                            wn[:, r * H + cchunk * 128: r * H + cchunk * 128 + 128],
                            id_sb[:])
                    nc.vector.tensor_copy(dst[:, cchunk * H:(cchunk + 1) * H], ps[:])

            # M^T rows chunk k = sum_oc W_out_nat[oc][:, k].T @ W_y^T[oc]
            # lhsT = W_out natural block [o,h'], rhs = W_y^T [o,h] -> out = M^T
            won = ini.tile([128, KC * H], F32, tag="wnat2")
            nc.sync.dma_start(
                won[:].rearrange("p (r i) -> p r i", r=KC),
                w_out.rearrange("(r p) i -> p r i", p=128))
            for k in range(KC):
                ps = ips.tile([128, 512], F32, tag="ips")
                for oc in range(KC):
                    nc.tensor.matmul(
                        ps[:],
                        won[:, oc * H + k * 128: oc * H + k * 128 + 128],
                        wyT[:, oc * H:(oc + 1) * H],
                        start=(oc == 0), stop=(oc == KC - 1))
                nc.vector.tensor_copy(mt_sb[:, k * H:(k + 1) * H], ps[:])

            # bias vectors as [128, KC]
            bi_sb = ini.tile([128, KC], F32, tag="bi")
            by_sb = ini.tile([128, KC], F32, tag="by")
            nc.sync.dma_start(bi_sb[:], b_in.rearrange("(c p) -> p c", p=128))
            nc.sync.dma_start(by_sb[:], b_y.rearrange("(c p) -> p c", p=128))
            nc.sync.dma_start(bout_sb[:], b_out.rearrange("(c p) -> p c", p=128))

            psb = ips.tile([128, 512], F32, tag="ips")
            for m in range(KC):
                for oc in range(KC):
                    nc.tensor.matmul(
                        psb[:, m:m + 1],
                        wyT[:, oc * H + m * 128: oc * H + m * 128 + 128],
                        bout_sb[:, oc:oc + 1],
                        start=(oc == 0), stop=(oc == KC - 1))
            nc.vector.tensor_add(bias_sb[:], psb[:, :KC], bi_sb[:])
            nc.vector.tensor_add(bias_sb[:], bias_sb[:], by_sb[:])

            # corr = W_y (ll - b_out): transpose ll, subtract, matmul
            ll_sb = ini.tile([BS, O], F32, tag="ll")
            nc.sync.dma_start(ll_sb[:], ll[:])
            llt = ini.tile([128, KC * BS], F32, tag="llt")
            pllt = ips.tile([128, 512], F32, tag="ips")
            for oc in range(KC):
                nc.tensor.transpose(
                    pllt[:, oc * BS:(oc + 1) * BS],
                    ll_sb[:, oc * 128:(oc + 1) * 128],
                    id_sb[:BS, :BS])
            for oc in range(KC):
                nc.vector.tensor_scalar_sub(
                    llt[:, oc * BS:(oc + 1) * BS],
                    pllt[:, oc * BS:(oc + 1) * BS],
                    bout_sb[:, oc:oc + 1])
            pcor = ips.tile([128, 512], F32, tag="ips")
            for m in range(KC):
                for oc in range(KC):
                    nc.tensor.matmul(
                        pcor[:, m * BS:(m + 1) * BS],
                        wyT[:, oc * H + m * 128: oc * H + m * 128 + 128],
                        llt[:, oc * BS:(oc + 1) * BS],
                        start=(oc == 0), stop=(oc == KC - 1))
            nc.vector.tensor_copy(corr_sb[:], pcor[:, :KC * BS])

            # ---- phase 1: c^T = W_in^T-matmul(E^T) + bias ----
            with (
                tc.tile_pool(name="p1", bufs=3) as p1,
                tc.tile_pool(name="p1ps", bufs=2, space="PSUM") as p1ps,
            ):
                ntiles = BT // 128
                for i in range(ntiles):
                    et_nat = p1.tile([128, I], F32, tag="enat")
                    nc.sync.dma_start(et_nat[:], emb[i * 128:(i + 1) * 128, :])
                    pst = p1ps.tile([128, 512], F32, tag="ptp")
                    for j in range(KC):
                        nc.tensor.transpose(
                            pst[:, j * 128:(j + 1) * 128],
                            et_nat[:, j * 128:(j + 1) * 128],
                            id_sb[:])
                    ett = p1.tile([128, 512], F32, tag="ett")
                    nc.vector.tensor_copy(ett[:], pst[:])
                    psc = p1ps.tile([128, 512], F32, tag="pc")
                    for m in range(KC):
                        for k in range(KC):
                            nc.tensor.matmul(
                                psc[:, m * 128:(m + 1) * 128],
                                winT[:, k * H + m * 128: k * H + m * 128 + 128],
                                ett[:, k * 128:(k + 1) * 128],
                                start=(k == 0), stop=(k == KC - 1))
                    for m in range(KC):
                        nc.scalar.activation(
                            c_sb[:, m * BT + i * 128: m * BT + i * 128 + 128],
                            psc[:, m * 128:(m + 1) * 128],
                            mybir.ActivationFunctionType.Identity,
                            bias=bias_sb[:, m:m + 1])

        # t = 0 fix: c_0 += corr - (W_y b_out part of bias)  [corr already has -W_y b_out]
        for m in range(KC):
            nc.vector.tensor_add(
                c_v[:, m, :, 0], c_v[:, m, :, 0], corr_sb[:, m * BS:(m + 1) * BS])

        # ---- phase 2: the scan ----
        with (
            tc.tile_pool(name="scan", bufs=3) as sp,
            tc.tile_pool(name="scan_ps", bufs=2, space="PSUM") as sps,
        ):
            ta = sp.tile([128, KC * BS], F16, tag="ta")
            nc.scalar.activation(h_v[:, :, :, 0], c_v[:, :, :, 0],
                                 mybir.ActivationFunctionType.Tanh)
            nc.vector.tensor_copy(
                ta[:].rearrange("p (m b) -> p m b", m=KC), h_v[:, :, :, 0])

            for t in range(1, T):
                ps = sps.tile([128, KC * BS], F32, tag="sps")
                for m in range(KC):
                    for k in range(KC):
                        nc.tensor.matmul(
                            ps[:, m * BS:(m + 1) * BS],
                            mt_sb[:, k * H + m * 128: k * H + m * 128 + 128],
                            ta[:, k * BS:(k + 1) * BS],
                            start=(k == 0), stop=(k == KC - 1))
                a_sb = sp.tile([128, KC * BS], F32, tag="a")
                nc.vector.tensor_add(
                    a_sb[:].rearrange("p (m b) -> p m b", m=KC),
                    ps[:].rearrange("p (m b) -> p m b", m=KC),
                    c_v[:, :, :, t])
                nc.scalar.activation(
                    h_v[:, :, :, t],
                    a_sb[:].rearrange("p (m b) -> p m b", m=KC),
                    mybir.ActivationFunctionType.Tanh)
                ta = sp.tile([128, KC * BS], F16, tag="ta")
                nc.vector.tensor_copy(
                    ta[:].rearrange("p (m b) -> p m b", m=KC), h_v[:, :, :, t])

        # ---- phase 3: y^T = W_out^T-matmul(h^T) + b_out; un-transpose h, y ----
        with (
            tc.tile_pool(name="p3", bufs=3) as p3,
            tc.tile_pool(name="p3ps", bufs=2, space="PSUM") as p3ps,
        ):
            for b in range(BS):
                y_sb = p3.tile([128, KC * T], F32, tag="ysb")
                for m in range(KC):
                    psy = p3ps.tile([128, T], F32, tag="py")
                    for k in range(KC):
                        nc.tensor.matmul(
                            psy[:],
                            woutT[:, k * H + m * 128: k * H + m * 128 + 128],
                            hT_sb[:, k * BT + b * T:(k * BT + b * T) + T],
                            start=(k == 0), stop=(k == KC - 1))
                    nc.scalar.activation(
                        y_sb[:, m * T:(m + 1) * T], psy[:],
                        mybir.ActivationFunctionType.Identity,
                        bias=bout_sb[:, m:m + 1])
                for s in range(KC):
                    pst = p3ps.tile([128, 512], F32, tag="pyt")
                    for m in range(KC):
                        nc.tensor.transpose(
                            pst[:, m * 128:(m + 1) * 128],
                            y_sb[:, m * T + s * 128: m * T + s * 128 + 128],
                            id_sb[:])
                    ynat = p3.tile([128, 512], F32, tag="ynat")
                    nc.vector.tensor_copy(ynat[:], pst[:])
                    nc.sync.dma_start(
                        y_out[b, s * 128:(s + 1) * 128, :], ynat[:])
                for s in range(KC):
                    psh = p3ps.tile([128, 512], F32, tag="pht")
                    for m in range(KC):
                        nc.tensor.transpose(
                            psh[:, m * 128:(m + 1) * 128],
                            hT_sb[:, m * BT + b * T + s * 128:
                                  m * BT + b * T + s * 128 + 128],
                            id_sb[:])
                    hnat = p3.tile([128, 512], F32, tag="hnat")
                    nc.vector.tensor_copy(hnat[:], psh[:])
                    nc.sync.dma_start(
                        h_out[b, s * 128:(s + 1) * 128, :], hnat[:])

    nc.compile()
    return nc


_NC_CACHE = None


def kernel(embeddings, last_logits, W_in, b_in, W_y, b_y, W_out, b_out):
    global LAST_RESULTS, _NC_CACHE
    if _NC_CACHE is None:
        _NC_CACHE = _build_nc()
    nc = _NC_CACHE

    embeddings = np.ascontiguousarray(np.asarray(embeddings, np.float32))
    last_logits = np.ascontiguousarray(np.asarray(last_logits, np.float32))
    shared = {
        "w_in": np.ascontiguousarray(np.asarray(W_in, np.float32)),
        "b_in": np.ascontiguousarray(np.asarray(b_in, np.float32)),
        "w_y": np.ascontiguousarray(np.asarray(W_y, np.float32)),
        "b_y": np.ascontiguousarray(np.asarray(b_y, np.float32)),
        "w_out": np.ascontiguousarray(np.asarray(W_out, np.float32)),
        "b_out": np.ascontiguousarray(np.asarray(b_out, np.float32)),
        "ident": np.eye(128, dtype=np.float32),
    }
    in_maps = []
    for c in range(NCORES):
        sl = slice(c * BS, (c + 1) * BS)
        in_maps.append({
            "emb": embeddings[sl].reshape(BT, I).copy(),
            "ll": last_logits[sl].copy(),
            **shared,
        })

    res = run_bass_kernel_spmd(
        nc, in_maps, list(range(NCORES)),
        trace=bool(os.environ.get("BASS_TRACE")))
    LAST_RESULTS = res

    h = np.concatenate([r["h_out"] for r in res.results], axis=0)
    y = np.concatenate([r["y_out"] for r in res.results], axis=0)
    y_last = np.ascontiguousarray(y[:, -1, :])
    return h, y, y_last
